# revision 24
# baseline (speedup 1.0000x reference)
"""Multi-head attention (S=2048, B=2, D=1024, H=16) on 8 Trainium2 NeuronCores.

Sharding: tensor-parallel over heads. Each core computes 2 heads end-to-end
(QKV projections restricted to its 128 output dims, attention, and the
row-parallel slice of the output projection). The host sums the 8 partial
outputs (row-parallel Wo ==> partial sums) and adds bo.

On-device compute is fp16 with fp32 PSUM accumulation. The softmax scale
folds into the exp activation; the softmax denominator comes for free from a
ones-column appended to V. The kernel is one software-pipelined stream over
(batch, query-chunk, key-block-pair) steps: the scalar engine's exp stream
(the hard ~138us floor) starts as soon as the first K/Q chunks land and never
drains across chunk or batch boundaries. DMA priority is arranged so the
4MB the stream start depends on (xk/xq first halves + Wk/Wq) lands first;
the last query chunk is split in two 256-wide sub-chunks to shorten the
serial normalize->outproj->writeback tail.
"""

import math

import numpy as np

S, B, D, H = 2048, 2, 1024, 16
DK = D // H  # 64
NCORES = 8
HLOC = H // NCORES        # heads per core = 2
DLOC = HLOC * DK          # local output dims per core = 128
T = S * B                 # tokens = 4096
KT = D // 128             # contraction tiles = 8
NQC = S // 512            # query chunks per batch = 4
NKB = S // 128            # key blocks = 16
NTT = S // 128            # token tiles per batch = 16
SCALE = 1.0 / math.sqrt(DK)

_prog_cache = {}


def _build(masked: bool):
    import concourse.mybir as mybir
    import concourse.tile as tile
    from concourse import bacc

    f16 = mybir.dt.float16
    f32 = mybir.dt.float32
    EXP = mybir.ActivationFunctionType.Exp
    MUL = mybir.AluOpType.mult
    ADD = mybir.AluOpType.add

    nc = bacc.Bacc("TRN2", target_bir_lowering=False, debug=False)

    def din(name, shape, dt=f16):
        return nc.dram_tensor(name, shape, dt, kind="ExternalInput").ap()

    xq = din("xq", [D, B, S])          # query^T
    xk = din("xk", [D, B, S])          # key^T
    xv = din("xv", [D, B, S])          # value^T
    # per-core projection weights, host-prearranged to [p, kt*m] so the
    # DMA is 128 partitions x 2KB contiguous (wq_arr[p, kt, m] = Wq[hs+m, kt*128+p])
    wq = din("wq", [128, KT * DLOC])
    wk = din("wk", [128, KT * DLOC])
    wv = din("wv", [128, KT * DLOC])
    wo = din("wo", [DLOC, D])          # Wo[:, hs:hs+128].T
    bq = din("bq", [DLOC], f32)
    bk = din("bk", [DLOC], f32)
    bv = din("bv", [DLOC], f32)
    mb = din("mb", [S], f32)           # additive mask bias per key (0 / -1e30)
    out = nc.dram_tensor("out", [S, B, D], f16, kind="ExternalOutput").ap()

    with tile.TileContext(nc) as tc:
        with (
            tc.tile_pool(name="wsb", bufs=1) as wsb,
            tc.tile_pool(name="xsb", bufs=8) as xsb,
            tc.tile_pool(name="qkv", bufs=1) as qkv,
            tc.tile_pool(name="esb", bufs=6) as esb,
            tc.tile_pool(name="nrm", bufs=3) as nrm,
            tc.tile_pool(name="osb", bufs=4) as osb,
            tc.tile_pool(name="pj", bufs=2, space="PSUM") as pj,
            tc.tile_pool(name="psc", bufs=2, space="PSUM") as psc,
            tc.tile_pool(name="pcx", bufs=1, space="PSUM") as pcx,
        ):
            # ---- warmup fodder: no DMA dependency; long enough to carry the
            # PE through its clock ramp and into the first projections ------
            junk = wsb.tile([128, 512], f16, tag="junk")
            nc.vector.memset(junk, 0.0)
            # short: a long full-rate warmup train saturates SBUF and stalls
            # the input DMA streams this phase is bandwidth-bound on
            for wu in range(4):
                jp = pj.tile([128, 512], f32, tag="pj", name="jp")
                nc.tensor.matmul(jp, junk[:, 0:128], junk,
                                 start=True, stop=True)

            # ---- weights: stream-critical ones first (Wk/Wq + biases), then
            # xq half 0 rides the scalar queue ahead of the remaining weights
            w_sb = {n: wsb.tile([128, KT, DLOC], f16, tag=n, name=n)
                    for n in ("wq", "wk", "wv")}
            bq_sb = wsb.tile([DLOC, 1], f32, tag="bq")
            bk_sb = wsb.tile([DLOC, 1], f32, tag="bk")
            bv_row = wsb.tile([1, DLOC], f32, tag="bv_row")
            bv_bc = wsb.tile([128, DLOC], f32, tag="bv_bc")
            wo_sb = wsb.tile([DLOC, D], f16, tag="wo")
            mb_sb = wsb.tile([128, NKB], f32, tag="mb")

            def wdma(t, ap):
                nc.scalar.dma_start(out=t, in_=ap)

            wdma(w_sb["wk"], wk.rearrange("p (kt m) -> p kt m", kt=KT))
            wdma(w_sb["wq"], wq.rearrange("p (kt m) -> p kt m", kt=KT))
            wdma(bk_sb, bk.unsqueeze(1))
            wdma(bq_sb, bq.unsqueeze(1))
            wdma(bv_row, bv.unsqueeze(0))
            wdma(w_sb["wv"], wv.rearrange("p (kt m) -> p kt m", kt=KT))
            # broadcast first in the gpsimd queue so V-prep isn't gated on
            # the xv stream finishing
            nc.gpsimd.partition_broadcast(bv_bc, bv_row)

            # persistent per-batch activations
            qT = [qkv.tile([DLOC, S], f16, tag=f"qT{b}", name=f"qT{b}") for b in range(B)]
            kT = [qkv.tile([DLOC, S], f16, tag=f"kT{b}", name=f"kT{b}") for b in range(B)]
            # V per (head, key-block): [keys=128, 65] with ones in col 64
            vv = [qkv.tile([128, HLOC, NKB, 68], f16, tag=f"vv{b}", name=f"vv{b}") for b in range(B)]
            for b in range(B):
                nc.vector.memset(vv[b][:, :, :, 64:65], 1.0)
            ctxn = [qkv.tile([DLOC, S], f16, tag=f"ctxn{b}", name=f"ctxn{b}") for b in range(B)]

            def load_x_half(ts, ap, b, eng, i):
                half = S // 2
                for kt in range(KT):
                    eng.dma_start(
                        out=ts[kt][:, i * half:(i + 1) * half],
                        in_=ap[kt * 128:(kt + 1) * 128, b,
                               i * half:(i + 1) * half])

            def xtiles(name):
                return [xsb.tile([128, S], f16, tag=f"x{name}", name=f"x{name}{kt}")
                        for kt in range(KT)]

            # b=0 loads. Per-core DMA-in bandwidth is shared across queues, so
            # everything goes on ONE queue (sync) in exact consumption order;
            # parallel queues would only starve the stream-gating prefix.
            xk_t = xtiles("k")
            xv_t = xtiles("v")
            xq_t = xtiles("q")
            load_x_half(xk_t, xk, 0, nc.sync, 0)
            load_x_half(xq_t, xq, 0, nc.sync, 0)
            load_x_half(xv_t, xv, 0, nc.sync, 0)
            load_x_half(xk_t, xk, 0, nc.sync, 1)
            load_x_half(xv_t, xv, 0, nc.sync, 1)
            load_x_half(xq_t, xq, 0, nc.sync, 1)
            nc.gpsimd.dma_start(out=wo_sb, in_=wo)
            nc.gpsimd.dma_start(out=mb_sb,
                                in_=mb.rearrange("(kb p) -> p kb", p=128))

            def proj_qk_chunk(b, which, xt, qc):
                w, bias, dst = (("wq", bq_sb, qT) if which == "q"
                                else ("wk", bk_sb, kT))
                ps = pj.tile([DLOC, 512], f32, tag="pj", name="ps")
                sl = slice(qc * 512, (qc + 1) * 512)
                for kt in range(KT):
                    nc.tensor.matmul(ps, w_sb[w][:, kt, :], xt[kt][:, sl],
                                     start=(kt == 0), stop=(kt == KT - 1))
                nc.vector.tensor_scalar(out=dst[b][:, sl], in0=ps,
                                        scalar1=bias, scalar2=None, op0=ADD)

            def proj_v_tt(b, xt, tts):
                for tt in tts:
                    ps = pj.tile([128, DLOC], f32, tag="pj", name="ps")
                    sl = slice(tt * 128, (tt + 1) * 128)
                    for kt in range(KT):
                        nc.tensor.matmul(ps, xt[kt][:, sl], w_sb["wv"][:, kt, :],
                                         start=(kt == 0), stop=(kt == KT - 1))
                    for h in range(HLOC):
                        nc.vector.tensor_tensor(
                            out=vv[b][:, h, tt, 0:64],
                            in0=ps[:, h * 64:(h + 1) * 64],
                            in1=bv_bc[:, h * 64:(h + 1) * 64], op=ADD)

            def outproj_tt(b, tts, tail=False):
                for n, tt in enumerate(tts):
                    tsl = slice(tt * 128, (tt + 1) * 128)
                    for eh in range(2):
                        po = pj.tile([128, 512], f32, tag="pj", name="po")
                        nc.tensor.matmul(po, ctxn[b][:, tsl],
                                         wo_sb[:, eh * 512:(eh + 1) * 512],
                                         start=True, stop=True)
                        oc = osb.tile([128, 512], f16, tag="oc", name="oc")
                        if tail and (n + eh) % 2 == 1:
                            nc.scalar.copy(oc, po)
                        else:
                            nc.vector.tensor_copy(oc, po)
                        deng = nc.gpsimd if eh == 0 else nc.sync
                        deng.dma_start(
                            out=out[tsl, b, eh * 512:(eh + 1) * 512], in_=oc)

            # ---- the attention stream -------------------------------------
            # steps: chunks 0-6 are full 512-wide query chunks; the last
            # query chunk (b=1, qc=3) is split into two 256-wide sub-chunks
            # to shorten the end-of-kernel serial tail.
            CH = [(b, qc, 0, 512) for b in range(B) for qc in range(NQC)][:-1]
            CH += [(1, 3, 0, 256), (1, 3, 256, 256)]
            NCH = len(CH)               # 9 chunks
            NST = NCH * 8               # 72 score-steps
            state = {}                  # chunk -> pctx tiles
            psq = {}                    # step  -> psco tiles

            def qslice(c):
                b, qc, qo, qw = CH[c]
                return slice(qc * 512 + qo, qc * 512 + qo + qw)

            def scores_j(j):
                c, kbp = divmod(j, 8)
                b, qc, qo, qw = CH[c]
                if kbp == 0:
                    state[c] = [pcx.tile([65, qw], f32, tag=f"cx{h}", name=f"cx{h}")
                                for h in range(HLOC)]
                qsl = qslice(c)
                psco = [psc.tile([128, 2 * qw], f32, tag="sc", name="sc")
                        for _ in range(HLOC)]
                for i in range(2):
                    kb = kbp * 2 + i
                    ksl = slice(kb * 128, (kb + 1) * 128)
                    for h in range(HLOC):
                        hsl = slice(h * 64, (h + 1) * 64)
                        nc.tensor.matmul(
                            psco[h][:, i * qw:(i + 1) * qw],
                            kT[b][hsl, ksl], qT[b][hsl, qsl],
                            start=True, stop=True,
                            tile_position=(h * 64, 0))
                psq[j] = psco

            def exp_ctx_j(j):
                c, kbp = divmod(j, 8)
                b, qc, qo, qw = CH[c]
                psco = psq.pop(j)
                pctx = state[c]
                escore = {}
                for h in range(HLOC):
                    et = esb.tile([128, 1024], f16, tag="e", name="et")
                    if masked:
                        for i in range(2):
                            kb = kbp * 2 + i
                            nc.scalar.activation(
                                et[:, i * qw:(i + 1) * qw],
                                psco[h][:, i * qw:(i + 1) * qw],
                                EXP, bias=mb_sb[:, kb:kb + 1], scale=SCALE)
                    else:
                        nc.scalar.activation(et[:, 0:2 * qw], psco[h],
                                             EXP, scale=SCALE)
                    escore[h] = et
                for i in range(2):
                    kb = kbp * 2 + i
                    for h in range(HLOC):
                        nc.tensor.matmul(
                            pctx[h], vv[b][:, h, kb, 0:65],
                            escore[h][:, i * qw:(i + 1) * qw],
                            start=(kb == 0), stop=(kb == NKB - 1))

            def normalize(c):
                b, qc, qo, qw = CH[c]
                qsl = qslice(c)
                # interleave the two heads' chains so the gpsimd broadcast of
                # h0 overlaps the DVE reciprocal of h1
                rls = []
                for h in range(HLOC):
                    cl = nrm.tile([1, qw], f32, tag="cl", name="cl")
                    nc.vector.tensor_copy(cl, state[c][h][64:65, :])
                    # reciprocal_approx_fast requires base partition 0 input
                    rl = nrm.tile([1, qw], f32, tag="rl", name="rl")
                    nc.vector.reciprocal_approx_fast(rl, cl)
                    rls.append(rl)
                bcs = []
                for h in range(HLOC):
                    rl_bc = nrm.tile([64, qw], f32, tag="rlb", name="rlb")
                    nc.gpsimd.partition_broadcast(rl_bc, rls[h])
                    bcs.append(rl_bc)
                for h in range(HLOC):
                    hsl = slice(h * 64, (h + 1) * 64)
                    nc.vector.tensor_tensor(out=ctxn[b][hsl, qsl],
                                            in0=state[c][h][0:64, :], in1=bcs[h],
                                            op=MUL)

            # pre-stream PE work, timed to land right as the DMAs complete
            proj_qk_chunk(0, "k", xk_t, 0)
            proj_qk_chunk(0, "q", xq_t, 0)

            x2 = {}
            sched = {
                2: [lambda: proj_qk_chunk(0, "k", xk_t, 1),
                    lambda: proj_v_tt(0, xv_t, range(2, 6))],
                3: [lambda: proj_qk_chunk(0, "q", xq_t, 1),
                    lambda: proj_qk_chunk(0, "k", xk_t, 2)],
                4: [lambda: proj_v_tt(0, xv_t, range(6, 10))],
                5: [lambda: proj_qk_chunk(0, "k", xk_t, 3)],
                6: [lambda: proj_v_tt(0, xv_t, range(10, 14))],
                7: [lambda: proj_v_tt(0, xv_t, range(14, 16))],
                8: [lambda: x2.update(k=xtiles("k")),
                    lambda: load_x_half(x2["k"], xk, 1, nc.sync, 0)],
                9: [lambda: load_x_half(x2["k"], xk, 1, nc.sync, 1)],
                10: [lambda: proj_qk_chunk(0, "q", xq_t, 2)],
                12: [lambda: proj_qk_chunk(1, "k", x2["k"], 0)],
                13: [lambda: proj_qk_chunk(0, "q", xq_t, 3)],
                14: [lambda: proj_qk_chunk(1, "k", x2["k"], 1)],
                15: [lambda: proj_qk_chunk(1, "k", x2["k"], 2)],
                16: [lambda: (proj_qk_chunk(1, "k", x2["k"], 3),
                              x2.update(q=xtiles("q")),
                              load_x_half(x2["q"], xq, 1, nc.sync, 0))],
                17: [lambda: x2.update(v=xtiles("v")),
                     lambda: load_x_half(x2["v"], xv, 1, nc.sync, 0)],
                18: [lambda: load_x_half(x2["q"], xq, 1, nc.sync, 1)],
                19: [lambda: load_x_half(x2["v"], xv, 1, nc.sync, 1)],
                21: [lambda: proj_qk_chunk(1, "q", x2["q"], 0)],
                22: [lambda: proj_v_tt(1, x2["v"], range(0, 4))],
                23: [lambda: proj_v_tt(1, x2["v"], range(4, 8))],
                24: [lambda: proj_v_tt(1, x2["v"], range(8, 12))],
                25: [lambda: proj_v_tt(1, x2["v"], range(12, 16))],
                27: [lambda: proj_qk_chunk(1, "q", x2["q"], 1)],
                34: [lambda: outproj_tt(0, [0, 1])],
                35: [lambda: outproj_tt(0, [2, 3])],
                36: [lambda: outproj_tt(0, [4, 5])],
                37: [lambda: outproj_tt(0, [6, 7])],
                38: [lambda: outproj_tt(0, [8, 9])],
                39: [lambda: outproj_tt(0, [10, 11])],
                40: [lambda: proj_qk_chunk(1, "q", x2["q"], 2)],
                42: [lambda: outproj_tt(0, [12, 13])],
                43: [lambda: outproj_tt(0, [14, 15])],
                44: [lambda: outproj_tt(1, [0, 1])],
                45: [lambda: outproj_tt(1, [2, 3])],
                48: [lambda: proj_qk_chunk(1, "q", x2["q"], 3)],
                50: [lambda: outproj_tt(1, [4, 5])],
                51: [lambda: outproj_tt(1, [6, 7])],
                58: [lambda: outproj_tt(1, [8, 9])],
                59: [lambda: outproj_tt(1, [10, 11])],
                67: [lambda: outproj_tt(1, [12, 13])],
            }

            scores_j(0)
            scores_j(1)
            proj_v_tt(0, xv_t, [0, 1])
            # injects run BEFORE scores(j): producers injected at slot j must
            # precede their same-engine consumers in queue order
            for j in range(2, NST + 2):
                for th in sched.get(j, ()):
                    th()
                if j < NST:
                    scores_j(j)
                exp_ctx_j(j - 2)
                c, kbp = divmod(j - 2, 8)
                if kbp == 7:
                    normalize(c)

            outproj_tt(1, [14, 15], tail=True)

    nc.compile()
    return nc


def _get_prog(masked: bool):
    key = masked
    if key not in _prog_cache:
        _prog_cache[key] = _build(masked)
    return _prog_cache[key]


def kernel(query, key, value, mask, Wq, bq, Wk, bk, Wv, bv, Wo, bo):
    from concourse.bass_utils import run_bass_kernel_spmd

    query = np.asarray(query)
    key = np.asarray(key)
    value = np.asarray(value)
    mask = np.asarray(mask)
    Wq, bq = np.asarray(Wq), np.asarray(bq)
    Wk, bk = np.asarray(Wk), np.asarray(bk)
    Wv, bv = np.asarray(Wv), np.asarray(bv)
    Wo, bo = np.asarray(Wo), np.asarray(bo)

    masked = not bool(mask.all())
    nc = _get_prog(masked)

    def t16(x):  # [S, B, D] -> contiguous [D, B, S] fp16
        return np.ascontiguousarray(x.transpose(2, 1, 0).astype(np.float16))

    def warr(W, hs):  # [128, KT*128]: row p = concat_kt W[hs+m, kt*128+p]
        wt = W[hs:hs + DLOC, :].T.astype(np.float16)       # [kt*128+p, m]
        return np.ascontiguousarray(
            wt.reshape(KT, 128, DLOC).transpose(1, 0, 2).reshape(128, KT * DLOC))

    xq, xk, xv = t16(query), t16(key), t16(value)
    mb = np.where(mask.reshape(S), 0.0, -1e30).astype(np.float32)

    in_maps = []
    for c in range(NCORES):
        hs = c * DLOC
        in_maps.append({
            "xq": xq, "xk": xk, "xv": xv,
            "wq": warr(Wq, hs),
            "wk": warr(Wk, hs),
            "wv": warr(Wv, hs),
            "wo": np.ascontiguousarray(Wo[:, hs:hs + DLOC].T.astype(np.float16)),
            "bq": bq[hs:hs + DLOC].astype(np.float32),
            "bk": bk[hs:hs + DLOC].astype(np.float32),
            "bv": bv[hs:hs + DLOC].astype(np.float32),
            "mb": mb,
        })

    res = run_bass_kernel_spmd(nc, in_maps, core_ids=list(range(NCORES)))
    acc = res.results[0]["out"].astype(np.float64)
    for c in range(1, NCORES):
        acc += res.results[c]["out"]
    acc += bo.astype(np.float64)
    return acc.astype(np.float32)


# revision 25
# speedup vs baseline: 1.0086x; 1.0086x over previous
"""Multi-head attention (S=2048, B=2, D=1024, H=16) on 8 Trainium2 NeuronCores.

Sharding: tensor-parallel over heads. Each core computes 2 heads end-to-end
(QKV projections restricted to its 128 output dims, attention, and the
row-parallel slice of the output projection). The host sums the 8 partial
outputs (row-parallel Wo ==> partial sums) and adds bo.

On-device compute is fp16 with fp32 PSUM accumulation. The softmax scale
folds into the exp activation; the softmax denominator comes for free from a
ones-column appended to V. The kernel is one software-pipelined stream over
(batch, query-chunk, key-block-pair) steps: the scalar engine's exp stream
(the hard ~138us floor) starts as soon as the first K/Q chunks land and never
drains across chunk or batch boundaries. DMA priority is arranged so the
4MB the stream start depends on (xk/xq first halves + Wk/Wq) lands first;
the last query chunk is split in two 256-wide sub-chunks to shorten the
serial normalize->outproj->writeback tail.
"""

import math

import numpy as np

S, B, D, H = 2048, 2, 1024, 16
DK = D // H  # 64
NCORES = 8
HLOC = H // NCORES        # heads per core = 2
DLOC = HLOC * DK          # local output dims per core = 128
T = S * B                 # tokens = 4096
KT = D // 128             # contraction tiles = 8
NQC = S // 512            # query chunks per batch = 4
NKB = S // 128            # key blocks = 16
NTT = S // 128            # token tiles per batch = 16
SCALE = 1.0 / math.sqrt(DK)

_prog_cache = {}


def _build(masked: bool):
    import concourse.mybir as mybir
    import concourse.tile as tile
    from concourse import bacc

    f16 = mybir.dt.float16
    f32 = mybir.dt.float32
    EXP = mybir.ActivationFunctionType.Exp
    MUL = mybir.AluOpType.mult
    ADD = mybir.AluOpType.add

    nc = bacc.Bacc("TRN2", target_bir_lowering=False, debug=False)

    def din(name, shape, dt=f16):
        return nc.dram_tensor(name, shape, dt, kind="ExternalInput").ap()

    xq = din("xq", [D, B, S])          # query^T
    xk = din("xk", [D, B, S])          # key^T
    xv = din("xv", [D, B, S])          # value^T
    # per-core projection weights, host-prearranged to [p, kt*m] so the
    # DMA is 128 partitions x 2KB contiguous (wq_arr[p, kt, m] = Wq[hs+m, kt*128+p])
    wq = din("wq", [128, KT * DLOC])
    wk = din("wk", [128, KT * DLOC])
    wv = din("wv", [128, KT * DLOC])
    wo = din("wo", [DLOC, D])          # Wo[:, hs:hs+128].T
    bq = din("bq", [DLOC], f32)
    bk = din("bk", [DLOC], f32)
    bv = din("bv", [DLOC], f32)
    mb = din("mb", [S], f32)           # additive mask bias per key (0 / -1e30)
    out = nc.dram_tensor("out", [S, B, D], f16, kind="ExternalOutput").ap()

    with tile.TileContext(nc) as tc:
        with (
            tc.tile_pool(name="wsb", bufs=1) as wsb,
            tc.tile_pool(name="xsb", bufs=8) as xsb,
            tc.tile_pool(name="qkv", bufs=1) as qkv,
            tc.tile_pool(name="esb", bufs=6) as esb,
            tc.tile_pool(name="nrm", bufs=3) as nrm,
            tc.tile_pool(name="osb", bufs=4) as osb,
            tc.tile_pool(name="pj", bufs=2, space="PSUM") as pj,
            tc.tile_pool(name="psc", bufs=2, space="PSUM") as psc,
            tc.tile_pool(name="pcx", bufs=1, space="PSUM") as pcx,
        ):
            # ---- warmup fodder: no DMA dependency; long enough to carry the
            # PE through its clock ramp and into the first projections ------
            junk = wsb.tile([128, 512], f16, tag="junk")
            nc.vector.memset(junk, 0.0)
            # long enough to carry the PE through its clock ramp, short
            # enough not to stall the bandwidth-bound input DMA streams
            for wu in range(12):
                jp = pj.tile([128, 512], f32, tag="pj", name="jp")
                nc.tensor.matmul(jp, junk[:, 0:128], junk,
                                 start=True, stop=True)

            # ---- weights: stream-critical ones first (Wk/Wq + biases), then
            # xq half 0 rides the scalar queue ahead of the remaining weights
            w_sb = {n: wsb.tile([128, KT, DLOC], f16, tag=n, name=n)
                    for n in ("wq", "wk", "wv")}
            bq_sb = wsb.tile([DLOC, 1], f32, tag="bq")
            bk_sb = wsb.tile([DLOC, 1], f32, tag="bk")
            bv_row = wsb.tile([1, DLOC], f32, tag="bv_row")
            bv_bc = wsb.tile([128, DLOC], f32, tag="bv_bc")
            wo_sb = wsb.tile([DLOC, D], f16, tag="wo")
            mb_sb = wsb.tile([128, NKB], f32, tag="mb")

            def wdma(t, ap):
                nc.scalar.dma_start(out=t, in_=ap)

            wdma(w_sb["wk"], wk.rearrange("p (kt m) -> p kt m", kt=KT))
            wdma(w_sb["wq"], wq.rearrange("p (kt m) -> p kt m", kt=KT))
            wdma(bk_sb, bk.unsqueeze(1))
            wdma(bq_sb, bq.unsqueeze(1))
            wdma(bv_row, bv.unsqueeze(0))
            wdma(w_sb["wv"], wv.rearrange("p (kt m) -> p kt m", kt=KT))
            # broadcast first in the gpsimd queue so V-prep isn't gated on
            # the xv stream finishing
            nc.gpsimd.partition_broadcast(bv_bc, bv_row)

            # persistent per-batch activations
            qT = [qkv.tile([DLOC, S], f16, tag=f"qT{b}", name=f"qT{b}") for b in range(B)]
            kT = [qkv.tile([DLOC, S], f16, tag=f"kT{b}", name=f"kT{b}") for b in range(B)]
            # V per (head, key-block): [keys=128, 65] with ones in col 64
            vv = [qkv.tile([128, HLOC, NKB, 68], f16, tag=f"vv{b}", name=f"vv{b}") for b in range(B)]
            for b in range(B):
                nc.vector.memset(vv[b][:, :, :, 64:65], 1.0)
            ctxn = [qkv.tile([DLOC, S], f16, tag=f"ctxn{b}", name=f"ctxn{b}") for b in range(B)]

            def load_x_half(ts, ap, b, eng, i):
                half = S // 2
                for kt in range(KT):
                    eng.dma_start(
                        out=ts[kt][:, i * half:(i + 1) * half],
                        in_=ap[kt * 128:(kt + 1) * 128, b,
                               i * half:(i + 1) * half])

            def xtiles(name):
                return [xsb.tile([128, S], f16, tag=f"x{name}", name=f"x{name}{kt}")
                        for kt in range(KT)]

            # b=0 loads. Per-core DMA-in bandwidth is shared across queues, so
            # everything goes on ONE queue (sync) in exact consumption order;
            # parallel queues would only starve the stream-gating prefix.
            xk_t = xtiles("k")
            xv_t = xtiles("v")
            xq_t = xtiles("q")
            load_x_half(xk_t, xk, 0, nc.sync, 0)
            load_x_half(xq_t, xq, 0, nc.sync, 0)
            load_x_half(xv_t, xv, 0, nc.sync, 0)
            load_x_half(xk_t, xk, 0, nc.sync, 1)
            load_x_half(xv_t, xv, 0, nc.sync, 1)
            load_x_half(xq_t, xq, 0, nc.sync, 1)
            nc.gpsimd.dma_start(out=wo_sb, in_=wo)
            nc.gpsimd.dma_start(out=mb_sb,
                                in_=mb.rearrange("(kb p) -> p kb", p=128))

            def proj_qk_chunk(b, which, xt, qc):
                w, bias, dst = (("wq", bq_sb, qT) if which == "q"
                                else ("wk", bk_sb, kT))
                ps = pj.tile([DLOC, 512], f32, tag="pj", name="ps")
                sl = slice(qc * 512, (qc + 1) * 512)
                for kt in range(KT):
                    nc.tensor.matmul(ps, w_sb[w][:, kt, :], xt[kt][:, sl],
                                     start=(kt == 0), stop=(kt == KT - 1))
                nc.vector.tensor_scalar(out=dst[b][:, sl], in0=ps,
                                        scalar1=bias, scalar2=None, op0=ADD)

            def proj_v_tt(b, xt, tts):
                for tt in tts:
                    ps = pj.tile([128, DLOC], f32, tag="pj", name="ps")
                    sl = slice(tt * 128, (tt + 1) * 128)
                    for kt in range(KT):
                        nc.tensor.matmul(ps, xt[kt][:, sl], w_sb["wv"][:, kt, :],
                                         start=(kt == 0), stop=(kt == KT - 1))
                    for h in range(HLOC):
                        nc.vector.tensor_tensor(
                            out=vv[b][:, h, tt, 0:64],
                            in0=ps[:, h * 64:(h + 1) * 64],
                            in1=bv_bc[:, h * 64:(h + 1) * 64], op=ADD)

            def outproj_tt(b, tts, tail=False):
                for n, tt in enumerate(tts):
                    tsl = slice(tt * 128, (tt + 1) * 128)
                    for eh in range(2):
                        po = pj.tile([128, 512], f32, tag="pj", name="po")
                        nc.tensor.matmul(po, ctxn[b][:, tsl],
                                         wo_sb[:, eh * 512:(eh + 1) * 512],
                                         start=True, stop=True)
                        oc = osb.tile([128, 512], f16, tag="oc", name="oc")
                        if tail and (n + eh) % 2 == 1:
                            nc.scalar.copy(oc, po)
                        else:
                            nc.vector.tensor_copy(oc, po)
                        deng = nc.gpsimd if eh == 0 else nc.sync
                        deng.dma_start(
                            out=out[tsl, b, eh * 512:(eh + 1) * 512], in_=oc)

            # ---- the attention stream -------------------------------------
            # steps: chunks 0-6 are full 512-wide query chunks; the last
            # query chunk (b=1, qc=3) is split into two 256-wide sub-chunks
            # to shorten the end-of-kernel serial tail.
            CH = [(b, qc, 0, 512) for b in range(B) for qc in range(NQC)][:-1]
            CH += [(1, 3, 0, 256), (1, 3, 256, 256)]
            NCH = len(CH)               # 9 chunks
            NST = NCH * 8               # 72 score-steps
            state = {}                  # chunk -> pctx tiles
            psq = {}                    # step  -> psco tiles

            def qslice(c):
                b, qc, qo, qw = CH[c]
                return slice(qc * 512 + qo, qc * 512 + qo + qw)

            def scores_j(j):
                c, kbp = divmod(j, 8)
                b, qc, qo, qw = CH[c]
                if kbp == 0:
                    state[c] = [pcx.tile([65, qw], f32, tag=f"cx{h}", name=f"cx{h}")
                                for h in range(HLOC)]
                qsl = qslice(c)
                psco = [psc.tile([128, 2 * qw], f32, tag="sc", name="sc")
                        for _ in range(HLOC)]
                for i in range(2):
                    kb = kbp * 2 + i
                    ksl = slice(kb * 128, (kb + 1) * 128)
                    for h in range(HLOC):
                        hsl = slice(h * 64, (h + 1) * 64)
                        nc.tensor.matmul(
                            psco[h][:, i * qw:(i + 1) * qw],
                            kT[b][hsl, ksl], qT[b][hsl, qsl],
                            start=True, stop=True,
                            tile_position=(h * 64, 0))
                psq[j] = psco

            def exp_ctx_j(j):
                c, kbp = divmod(j, 8)
                b, qc, qo, qw = CH[c]
                psco = psq.pop(j)
                pctx = state[c]
                escore = {}
                for h in range(HLOC):
                    et = esb.tile([128, 1024], f16, tag="e", name="et")
                    if masked:
                        for i in range(2):
                            kb = kbp * 2 + i
                            nc.scalar.activation(
                                et[:, i * qw:(i + 1) * qw],
                                psco[h][:, i * qw:(i + 1) * qw],
                                EXP, bias=mb_sb[:, kb:kb + 1], scale=SCALE)
                    else:
                        nc.scalar.activation(et[:, 0:2 * qw], psco[h],
                                             EXP, scale=SCALE)
                    escore[h] = et
                for i in range(2):
                    kb = kbp * 2 + i
                    for h in range(HLOC):
                        nc.tensor.matmul(
                            pctx[h], vv[b][:, h, kb, 0:65],
                            escore[h][:, i * qw:(i + 1) * qw],
                            start=(kb == 0), stop=(kb == NKB - 1))

            def normalize(c):
                b, qc, qo, qw = CH[c]
                qsl = qslice(c)
                # interleave the two heads' chains so the gpsimd broadcast of
                # h0 overlaps the DVE reciprocal of h1
                rls = []
                for h in range(HLOC):
                    cl = nrm.tile([1, qw], f32, tag="cl", name="cl")
                    nc.vector.tensor_copy(cl, state[c][h][64:65, :])
                    # reciprocal_approx_fast requires base partition 0 input
                    rl = nrm.tile([1, qw], f32, tag="rl", name="rl")
                    nc.vector.reciprocal_approx_fast(rl, cl)
                    rls.append(rl)
                bcs = []
                for h in range(HLOC):
                    rl_bc = nrm.tile([64, qw], f32, tag="rlb", name="rlb")
                    nc.gpsimd.partition_broadcast(rl_bc, rls[h])
                    bcs.append(rl_bc)
                for h in range(HLOC):
                    hsl = slice(h * 64, (h + 1) * 64)
                    nc.vector.tensor_tensor(out=ctxn[b][hsl, qsl],
                                            in0=state[c][h][0:64, :], in1=bcs[h],
                                            op=MUL)

            # pre-stream PE work, timed to land right as the DMAs complete
            proj_qk_chunk(0, "k", xk_t, 0)
            proj_qk_chunk(0, "q", xq_t, 0)

            x2 = {}
            sched = {
                2: [lambda: proj_qk_chunk(0, "k", xk_t, 1),
                    lambda: proj_v_tt(0, xv_t, range(2, 6))],
                3: [lambda: proj_qk_chunk(0, "q", xq_t, 1),
                    lambda: proj_qk_chunk(0, "k", xk_t, 2)],
                4: [lambda: proj_v_tt(0, xv_t, range(6, 10))],
                5: [lambda: proj_qk_chunk(0, "k", xk_t, 3)],
                6: [lambda: proj_v_tt(0, xv_t, range(10, 14))],
                7: [lambda: proj_v_tt(0, xv_t, range(14, 16))],
                8: [lambda: x2.update(k=xtiles("k")),
                    lambda: load_x_half(x2["k"], xk, 1, nc.sync, 0)],
                9: [lambda: load_x_half(x2["k"], xk, 1, nc.sync, 1)],
                10: [lambda: proj_qk_chunk(0, "q", xq_t, 2)],
                12: [lambda: proj_qk_chunk(1, "k", x2["k"], 0)],
                13: [lambda: proj_qk_chunk(0, "q", xq_t, 3)],
                14: [lambda: proj_qk_chunk(1, "k", x2["k"], 1)],
                15: [lambda: proj_qk_chunk(1, "k", x2["k"], 2)],
                16: [lambda: (proj_qk_chunk(1, "k", x2["k"], 3),
                              x2.update(q=xtiles("q")),
                              load_x_half(x2["q"], xq, 1, nc.sync, 0))],
                17: [lambda: x2.update(v=xtiles("v")),
                     lambda: load_x_half(x2["v"], xv, 1, nc.sync, 0)],
                18: [lambda: load_x_half(x2["q"], xq, 1, nc.sync, 1)],
                19: [lambda: load_x_half(x2["v"], xv, 1, nc.sync, 1)],
                21: [lambda: proj_qk_chunk(1, "q", x2["q"], 0)],
                22: [lambda: proj_v_tt(1, x2["v"], range(0, 4))],
                23: [lambda: proj_v_tt(1, x2["v"], range(4, 8))],
                24: [lambda: proj_v_tt(1, x2["v"], range(8, 12))],
                25: [lambda: proj_v_tt(1, x2["v"], range(12, 16))],
                27: [lambda: proj_qk_chunk(1, "q", x2["q"], 1)],
                34: [lambda: outproj_tt(0, [0, 1])],
                35: [lambda: outproj_tt(0, [2, 3])],
                36: [lambda: outproj_tt(0, [4, 5])],
                37: [lambda: outproj_tt(0, [6, 7])],
                38: [lambda: outproj_tt(0, [8, 9])],
                39: [lambda: outproj_tt(0, [10, 11])],
                40: [lambda: proj_qk_chunk(1, "q", x2["q"], 2)],
                42: [lambda: outproj_tt(0, [12, 13])],
                43: [lambda: outproj_tt(0, [14, 15])],
                44: [lambda: outproj_tt(1, [0, 1])],
                45: [lambda: outproj_tt(1, [2, 3])],
                48: [lambda: proj_qk_chunk(1, "q", x2["q"], 3)],
                50: [lambda: outproj_tt(1, [4, 5])],
                51: [lambda: outproj_tt(1, [6, 7])],
                58: [lambda: outproj_tt(1, [8, 9])],
                59: [lambda: outproj_tt(1, [10, 11])],
                67: [lambda: outproj_tt(1, [12, 13])],
            }

            scores_j(0)
            scores_j(1)
            proj_v_tt(0, xv_t, [0, 1])
            # injects run BEFORE scores(j): producers injected at slot j must
            # precede their same-engine consumers in queue order
            for j in range(2, NST + 2):
                for th in sched.get(j, ()):
                    th()
                if j < NST:
                    scores_j(j)
                exp_ctx_j(j - 2)
                c, kbp = divmod(j - 2, 8)
                if kbp == 7:
                    normalize(c)

            outproj_tt(1, [14, 15], tail=True)

    nc.compile()
    return nc


def _get_prog(masked: bool):
    key = masked
    if key not in _prog_cache:
        _prog_cache[key] = _build(masked)
    return _prog_cache[key]


def kernel(query, key, value, mask, Wq, bq, Wk, bk, Wv, bv, Wo, bo):
    from concourse.bass_utils import run_bass_kernel_spmd

    query = np.asarray(query)
    key = np.asarray(key)
    value = np.asarray(value)
    mask = np.asarray(mask)
    Wq, bq = np.asarray(Wq), np.asarray(bq)
    Wk, bk = np.asarray(Wk), np.asarray(bk)
    Wv, bv = np.asarray(Wv), np.asarray(bv)
    Wo, bo = np.asarray(Wo), np.asarray(bo)

    masked = not bool(mask.all())
    nc = _get_prog(masked)

    def t16(x):  # [S, B, D] -> contiguous [D, B, S] fp16
        return np.ascontiguousarray(x.transpose(2, 1, 0).astype(np.float16))

    def warr(W, hs):  # [128, KT*128]: row p = concat_kt W[hs+m, kt*128+p]
        wt = W[hs:hs + DLOC, :].T.astype(np.float16)       # [kt*128+p, m]
        return np.ascontiguousarray(
            wt.reshape(KT, 128, DLOC).transpose(1, 0, 2).reshape(128, KT * DLOC))

    xq, xk, xv = t16(query), t16(key), t16(value)
    mb = np.where(mask.reshape(S), 0.0, -1e30).astype(np.float32)

    in_maps = []
    for c in range(NCORES):
        hs = c * DLOC
        in_maps.append({
            "xq": xq, "xk": xk, "xv": xv,
            "wq": warr(Wq, hs),
            "wk": warr(Wk, hs),
            "wv": warr(Wv, hs),
            "wo": np.ascontiguousarray(Wo[:, hs:hs + DLOC].T.astype(np.float16)),
            "bq": bq[hs:hs + DLOC].astype(np.float32),
            "bk": bk[hs:hs + DLOC].astype(np.float32),
            "bv": bv[hs:hs + DLOC].astype(np.float32),
            "mb": mb,
        })

    res = run_bass_kernel_spmd(nc, in_maps, core_ids=list(range(NCORES)))
    acc = res.results[0]["out"].astype(np.float64)
    for c in range(1, NCORES):
        acc += res.results[c]["out"]
    acc += bo.astype(np.float64)
    return acc.astype(np.float32)
